# revision 41
# baseline (speedup 1.0000x reference)
"""Causal multi-head self-attention with RoPE on 8 NeuronCores.

Sharding: batch (4) x head-halves (2) -> 8 cores. Core c handles batch c//2,
heads [8*(c%2), 8*(c%2)+8). Fully software-pipelined: QKV projection chunks,
attention chunks and the output projection are interleaved in emission order
so the PE never starves while the scalar engine works through the softmax
exps. Output projection is row-sharded over Wo with a per-chunk pair
ReduceScatter writing disjoint row-shards of y on each core; the host
interleaves the shards.
"""

import numpy as np
import ml_dtypes

import concourse.bacc as bacc
import concourse.bass as bass
import concourse.mybir as mybir
from concourse.tile import TileContext
from concourse.bass_utils import run_bass_kernel_spmd

B, S, D, H = 4, 2048, 1024, 16
HL = 8          # heads per core
DK = 64         # head dim
NCORES = 8
DT = D // 128   # 8 contraction tiles over D
OT = HL * DK // 128   # 4 o-tiles for Q^T/K^T ([128, S] each, 2 heads per tile)
ST = S // 128   # 16 s-tiles
NCH = S // 512  # 4 sequence chunks of 512
# vaug col group per head pair: [v_e(64) | 1 | 0*63 | 1 | v_o(64)].
# The even PV matmul uses cols [0:65] -> psum partitions [0:65] (sums at 64);
# the odd uses cols [65:193] (a full 128-wide block at output base partition
# 0) -> sums land at partition 63, data at 64:128, all partition-aligned for
# the downstream normalization multiplies.
GW = 193

BF16 = mybir.dt.bfloat16
F32 = mybir.dt.float32
NEG = -1.0e9

_compiled = {}


def _build_nc():
    nc = bacc.Bacc("TRN2", target_bir_lowering=False, debug=False,
                   num_devices=NCORES)

    xT = nc.dram_tensor("xT", [D, S], BF16, kind="ExternalInput")
    wqT = nc.dram_tensor("wqT", [D, HL * DK], BF16, kind="ExternalInput")
    wkT = nc.dram_tensor("wkT", [D, HL * DK], BF16, kind="ExternalInput")
    wvT = nc.dram_tensor("wvT", [D, HL * DK], BF16, kind="ExternalInput")
    woT = nc.dram_tensor("woT", [HL * DK, D], BF16, kind="ExternalInput")
    cosT = nc.dram_tensor("cosT", [128, S], BF16, kind="ExternalInput")
    sinT = nc.dram_tensor("sinT", [128, S], BF16, kind="ExternalInput")
    swapT = nc.dram_tensor("swapT", [128, 128], BF16, kind="ExternalInput")
    maskT = nc.dram_tensor("maskT", [128, 128], BF16, kind="ExternalInput")
    eT = nc.dram_tensor("eT", [2, 128], mybir.dt.float16, kind="ExternalInput")
    y = nc.dram_tensor("y", [S // 2, D], BF16, kind="ExternalOutput")

    groups = [[0, 1], [2, 3], [4, 5], [6, 7]]

    with TileContext(nc) as tc:
        with (
            tc.tile_pool(name="big", bufs=1) as big,
            tc.tile_pool(name="qr", bufs=2) as qrp,
            tc.tile_pool(name="otp", bufs=2) as otp,
            tc.tile_pool(name="ptile", bufs=16) as ptile,
            tc.tile_pool(name="work", bufs=2) as work,
            tc.tile_pool(name="ps_s", bufs=2, space="PSUM") as ps_s,
            tc.tile_pool(name="ps_a", bufs=2, space="PSUM") as ps_a,
            tc.tile_pool(name="ps_o", bufs=2, space="PSUM") as ps_o,
            tc.tile_pool(name="dram", bufs=1, space="DRAM") as dram,
        ):
            # ---- input loads: priority order, spread over two queues ----
            # the first emitted work is the chunk-0 K projection + RoPE, so
            # wk, sin/cos/swap must land first on the scalar queue; xT leads
            # the sync queue.
            w_sb = {"q": [], "k": [], "v": []}
            for k in range(DT):
                t = big.tile([128, HL * DK], BF16, tag=f"wk{k}", name=f"wk{k}")
                nc.scalar.dma_start(out=t[:], in_=wkT[128 * k:128 * (k + 1), :])
                w_sb["k"].append(t)
            xT_sb = []
            for k in range(DT):
                t = big.tile([128, S], BF16, tag=f"xT{k}", name=f"xT{k}")
                nc.sync.dma_start(out=t[:], in_=xT[128 * k:128 * (k + 1), :])
                xT_sb.append(t)
            sin_sb = big.tile([128, S], BF16, tag="sin")
            nc.scalar.dma_start(out=sin_sb[:], in_=sinT[:])
            cos_sb = big.tile([128, S], BF16, tag="cos")
            nc.scalar.dma_start(out=cos_sb[:], in_=cosT[:])
            swap_sb = big.tile([128, 128], BF16, tag="swap")
            nc.scalar.dma_start(out=swap_sb[:], in_=swapT[:])
            for k in range(DT):
                t = big.tile([128, HL * DK], BF16, tag=f"wq{k}", name=f"wq{k}")
                nc.scalar.dma_start(out=t[:], in_=wqT[128 * k:128 * (k + 1), :])
                w_sb["q"].append(t)
            for k in range(DT):
                t = big.tile([128, HL * DK], BF16, tag=f"wv{k}", name=f"wv{k}")
                nc.sync.dma_start(out=t[:], in_=wvT[128 * k:128 * (k + 1), :])
                w_sb["v"].append(t)
            mask_sb = big.tile([128, 128], BF16, tag="mask")
            nc.scalar.dma_start(out=mask_sb[:], in_=maskT[:])
            e_sb = big.tile([2, 128], mybir.dt.float16, tag="eT")
            nc.scalar.dma_start(out=e_sb[:], in_=eT[:])
            woT_sb = []
            for k in range(OT):
                t = big.tile([128, D], BF16, tag=f"wo{k}", name=f"wo{k}")
                nc.sync.dma_start(out=t[:], in_=woT[128 * k:128 * (k + 1), :])
                woT_sb.append(t)

            # ---- persistent activation tiles ----
            krot_sb = [big.tile([128, S], BF16, tag=f"kr{t}", name=f"kr{t}")
                       for t in range(OT)]
            vaug_sb = [big.tile([128, OT * GW], BF16, tag=f"va{st}",
                                name=f"va{st}") for st in range(ST)]
            # ones / zero-pad columns of each 193-wide head-pair group
            for st in range(ST):
                va3 = vaug_sb[st][:].rearrange("p (g d) -> p g d", d=GW)
                nc.gpsimd.memset(va3[:, :, DK:DK + 1], 1.0)
                nc.gpsimd.memset(va3[:, :, DK + 1:DK + 64], 0.0)
                nc.gpsimd.memset(va3[:, :, DK + 64:DK + 65], 1.0)

            qr_tiles = {}   # (m, ot) -> tile [128, 512]
            oT_tiles = {}   # (m, ot) -> tile [128, 512]
            ypart = [dram.tile([512, D], BF16, tag=f"ypart{m}", name=f"ypart{m}")
                     for m in range(NCH)]

            # ---- quantum builders (deferred emission closures) ----
            def qk_quantum(wname, ot, n):
                # returns (chain_fn, tail_fn): projection chain, then RoPE
                ps_box = {}

                def chain():
                    ps = ps_a.tile([128, 512], F32, tag="psa", name=f"{wname}{ot}_{n}")
                    for k in range(DT):
                        nc.tensor.matmul(
                            ps[:],
                            lhsT=w_sb[wname][k][:, 128 * ot:128 * (ot + 1)],
                            rhs=xT_sb[k][:, 512 * n:512 * (n + 1)],
                            start=(k == 0), stop=(k == DT - 1),
                        )
                    sl = slice(512 * n, 512 * (n + 1))
                    u = work.tile([128, 512], BF16, tag="u", name="u")
                    nc.vector.tensor_mul(u[:], ps[:], sin_sb[:, sl])
                    t1 = work.tile([128, 512], BF16, tag="t1", name="t1")
                    nc.vector.tensor_mul(t1[:], ps[:], cos_sb[:, sl])
                    ps_box["u"], ps_box["t1"] = u, t1

                def tail():
                    sl = slice(512 * n, 512 * (n + 1))
                    ps2 = ps_a.tile([128, 512], F32, tag="psa", name=f"sw{wname}{ot}_{n}")
                    nc.tensor.matmul(ps2[:], lhsT=swap_sb[:], rhs=ps_box["u"][:],
                                     start=True, stop=True)
                    if wname == "q":
                        dst_t = qrp.tile([128, 512], BF16, tag=f"qr{ot}",
                                         name=f"qr{ot}_{n}")
                        qr_tiles[(n, ot)] = dst_t
                        dst = dst_t[:]
                    else:
                        dst = krot_sb[ot][:, sl]
                    nc.vector.tensor_add(dst, ps_box["t1"][:], ps2[:])

                return chain, tail

            def v_quantum(st):
                def chain():
                    ps = ps_a.tile([128, 512], F32, tag="psa", name=f"v{st}")
                    for k in range(DT):
                        nc.tensor.matmul(
                            ps[:],
                            lhsT=xT_sb[k][:, 128 * st:128 * (st + 1)],
                            rhs=w_sb["v"][k][:],
                            start=(k == 0), stop=(k == DT - 1),
                        )
                    va3 = vaug_sb[st][:].rearrange("p (g d) -> p g d", d=GW)
                    src3 = ps[:].rearrange("p (g d) -> p g d", d=128)
                    nc.vector.tensor_copy(va3[:, :, 0:DK], src3[:, :, 0:DK])
                    nc.vector.tensor_copy(va3[:, :, DK + 65:GW],
                                          src3[:, :, DK:128])
                return chain, None

            def proj_quantum(m, r2, nn):
                def chain():
                    i0 = 512 * m
                    r0 = 128 * r2
                    yp = ps_a.tile([128, 512], F32, tag="psa", name=f"yp{m}_{r2}_{nn}")
                    for k in range(OT):
                        nc.tensor.matmul(
                            yp[:],
                            lhsT=oT_tiles[(m, k)][:, r0:r0 + 128],
                            rhs=woT_sb[k][:, 512 * nn:512 * (nn + 1)],
                            start=(k == 0), stop=(k == OT - 1),
                        )
                    yst = work.tile([128, 512], BF16, tag="yst", name="yst")
                    nc.vector.tensor_copy(yst[:], yp[:])
                    nc.sync.dma_start(
                        out=ypart[m][r0:r0 + 128, 512 * nn:512 * (nn + 1)],
                        in_=yst[:])
                return chain, None

            yred = [dram.tile([256, D], BF16, tag=f"yred{m}", name=f"yred{m}")
                    for m in range(NCH)]

            def rs_quantum(m, piece):
                # each chunk's ReduceScatter is split in two halves so the
                # collective overlaps the projection compute. This core's
                # shard of piece p covers global rows 512m+256p + [0,128)
                # (even cores) or +[128,256) (odd cores); stored at
                # y[256m+128p : 256m+128p+128] and interleaved on the host.
                def chain():
                    sl_in = slice(256 * piece, 256 * piece + 256)
                    sl_red = slice(128 * piece, 128 * piece + 128)
                    sl_y = slice(256 * m + 128 * piece,
                                 256 * m + 128 * piece + 128)
                    nc.gpsimd.collective_compute(
                        "ReduceScatter", mybir.AluOpType.add,
                        replica_groups=groups,
                        ins=[ypart[m][sl_in, :].opt()],
                        outs=[yred[m][sl_red, :].opt()],
                    )
                    nc.sync.dma_start(out=y[sl_y, :], in_=yred[m][sl_red, :])
                return chain, None

            def kvq_quanta(m):
                qs = []
                for ot in range(OT):
                    qs.append(qk_quantum("k", ot, m))
                for ot in range(OT):
                    qs.append(qk_quantum("q", ot, m))
                for st in range(4 * m, 4 * m + 4):
                    qs.append(v_quantum(st))
                return qs

            def proj_quanta(m):
                qs = []
                for r2 in range(4):
                    for nn in range(2):
                        qs.append(proj_quantum(m, r2, nn))
                    if r2 == 1:
                        qs.append(rs_quantum(m, piece=0))
                qs.append(rs_quantum(m, piece=1))
                return qs

            # filler stream with one-step delayed tails (keeps PE fed while
            # DVE produces the rope operands)
            class Fillers:
                def __init__(self, quanta):
                    self.quanta = list(quanta)
                    self.pend = None
                    self.idx = 0

                def emit_one(self):
                    if self.idx < len(self.quanta):
                        chain, tail = self.quanta[self.idx]
                        self.idx += 1
                        chain()
                        if self.pend is not None:
                            self.pend()
                        self.pend = tail
                        return True
                    if self.pend is not None:
                        self.pend()
                        self.pend = None
                        return True
                    return False

                def flush(self):
                    while self.emit_one():
                        pass

            # ---- attention chunk ----
            def attn_chunk(m, fillers):
                i0 = 512 * m
                njb = 4 * (m + 1)
                total_slots = OT * njb
                n_fill = len(fillers.quanta) + 1
                acc = 0.0
                pace = max(0.0, (n_fill - OT)) / total_slots
                for tp in range(OT):
                    pTs = []
                    # PV-e chain tiles; emitted at lag 2 inside the QK loop so
                    # the PE has exp-independent work when the scalar engine
                    # lags
                    o_pse = ps_o.tile([128, 512], F32, tag="pso",
                                      name=f"oe{m}_{tp}")

                    def pv_e(jb):
                        dlt = max(0, 128 * jb - i0)
                        nc.tensor.matmul(
                            o_pse[0:DK + 1, dlt:512],
                            lhsT=vaug_sb[jb][:, GW * tp:GW * tp + DK + 1],
                            rhs=pTs[jb][:, dlt:512],
                            start=(jb == 0), stop=(jb == njb - 1),
                            skip_group_check=True,
                        )

                    for jb in range(njb):
                        j0 = 128 * jb
                        dlt = max(0, j0 - i0)
                        s_ps = ps_s.tile([128, 1024], F32, tag="sps",
                                         name=f"s{m}_{tp}_{jb}")
                        for half, po in ((0, 0), (1, DK)):
                            nc.tensor.matmul(
                                s_ps[:, 512 * half + dlt:512 * (half + 1)],
                                lhsT=krot_sb[tp][po:po + DK, j0:j0 + 128],
                                rhs=qr_tiles[(m, tp)][po:po + DK, dlt:512],
                                start=True, stop=True,
                            )
                        pT = ptile.tile([128, 1024], BF16, tag="pT")
                        nc.scalar.activation(
                            pT[:].rearrange("p (b f) -> p b f", b=2)[:, :, dlt:512],
                            s_ps[:].rearrange("p (b f) -> p b f", b=2)[:, :, dlt:512],
                            mybir.ActivationFunctionType.Exp, scale=0.125)
                        if j0 >= i0:
                            # zero the upper-triangle entries post-exp; keeps
                            # the DVE off the QK->exp critical path
                            p3 = pT[:].rearrange("p (b f) -> p b f", b=2)
                            nc.vector.tensor_mul(
                                p3[:, :, dlt:dlt + 128],
                                p3[:, :, dlt:dlt + 128],
                                mask_sb[:].rearrange("p (b f) -> p b f", b=1)
                                .broadcast_to([128, 2, 128]))
                        pTs.append(pT)
                        if jb >= 2:
                            pv_e(jb - 2)
                        acc += pace
                        while acc >= 1.0:
                            fillers.emit_one()
                            acc -= 1.0
                    pv_e(njb - 2)
                    pv_e(njb - 1)
                    o_pso = ps_o.tile([128, 512], F32, tag="pso",
                                      name=f"oo{m}_{tp}")
                    for jb in range(njb):
                        dlt = max(0, 128 * jb - i0)
                        nc.tensor.matmul(
                            o_pso[:, dlt:512],
                            lhsT=vaug_sb[jb][:, GW * tp + DK + 1:GW * (tp + 1)],
                            rhs=pTs[jb][:, 512 + dlt:1024],
                            start=(jb == 0), stop=(jb == njb - 1),
                        )
                    fillers.emit_one()
                    # normalization: stage the two sum rows to SBUF, fold the
                    # 2x512 sums into a [128, 8] view by DMA (DVE reciprocal
                    # cost scales with free-dim size only), reciprocal there,
                    # then broadcast across dims via a K=2 matmul against the
                    # selector E.
                    stg = work.tile([128, 512], F32, tag="stg", name="stg")
                    nc.scalar.copy(stg[DK:DK + 1, :], o_pse[DK:DK + 1, :])
                    nc.scalar.copy(stg[32:DK, :], o_pso[32:DK, :])
                    recw = work.tile([128, 8], F32, tag="recw", name="recw")
                    nc.gpsimd.dma_start(out=recw[:, 0:4],
                                        in_=stg[DK:DK + 1, :])
                    nc.gpsimd.dma_start(out=recw[:, 4:8],
                                        in_=stg[DK - 1:DK, :])
                    nc.vector.reciprocal(recw[:], recw[:])
                    recwh = work.tile([128, 8], mybir.dt.float16, tag="recwh",
                                      name="recwh")
                    nc.vector.tensor_copy(recwh[:], recw[:])
                    rec2h = work.tile([2, 512], mybir.dt.float16, tag="rec2h",
                                      name="rec2h")
                    nc.gpsimd.dma_start(out=rec2h[0:1, :], in_=recwh[:, 0:4])
                    nc.gpsimd.dma_start(out=rec2h[1:2, :], in_=recwh[:, 4:8])
                    rep_ps = ps_a.tile([128, 512], F32, tag="psa",
                                       name=f"rep{m}_{tp}")
                    nc.tensor.matmul(rep_ps[:], lhsT=e_sb[:], rhs=rec2h[:],
                                     start=True, stop=True)
                    rep_sb = work.tile([128, 512], F32, tag="rep", name="rep")
                    nc.vector.tensor_copy(rep_sb[:], rep_ps[:])
                    oT_t = otp.tile([128, 512], BF16, tag=f"oT{tp}",
                                    name=f"oT{tp}_{m}")
                    oT_tiles[(m, tp)] = oT_t
                    nc.vector.tensor_mul(oT_t[0:DK, :], o_pse[0:DK, :],
                                         rep_sb[0:DK, :])
                    nc.vector.tensor_mul(oT_t[DK:128, :], o_pso[DK:128, :],
                                         rep_sb[DK:128, :])
                fillers.flush()

            # ---- top-level schedule ----
            f0 = Fillers(kvq_quanta(0))
            f0.flush()
            sched = {
                0: kvq_quanta(1),
                1: kvq_quanta(2) + proj_quanta(0),
                2: kvq_quanta(3) + proj_quanta(1),
                3: proj_quanta(2),
            }
            for m in range(NCH):
                attn_chunk(m, Fillers(sched[m]))
            Fillers(proj_quanta(3)).flush()

    nc.compile()
    return nc


def _prep_inputs(x, Wq, Wk, Wv, Wo, cos_emb, sin_emb, token_positions):
    bf = ml_dtypes.bfloat16
    cos_g = np.asarray(cos_emb)[np.asarray(token_positions)]  # [S, DK]
    sin_g = np.asarray(sin_emb)[np.asarray(token_positions)]
    # [128, S]: partition p -> head-dim p % 64
    cosT = np.ascontiguousarray(np.tile(cos_g.T, (2, 1))).astype(bf)
    sinT = np.ascontiguousarray(np.tile(sin_g.T, (2, 1))).astype(bf)
    # rotate-half-interleaved as a matmul: rh = SWAP @ u (per 128-dim tile)
    swap = np.zeros((128, 128), np.float32)
    for j in range(64):
        swap[2 * j, 2 * j + 1] = -1.0
        swap[2 * j + 1, 2 * j] = 1.0
    swapT = np.ascontiguousarray(swap.T).astype(bf)
    # 0/1 causal mask for the diagonal 128x128 block in P^T=[j,i] layout,
    # applied multiplicatively after the exp
    jj = np.arange(128)[:, None]
    ii = np.arange(128)[None, :]
    maskT = np.where(ii >= jj, 1.0, 0.0).astype(bf)
    # selector for broadcasting the two per-head reciprocals across dims
    eT = np.zeros((2, 128), np.float16)
    eT[0, 0:64] = 1.0
    eT[1, 64:128] = 1.0

    in_maps = []
    for c in range(NCORES):
        b, hh = c // 2, c % 2
        cols = slice(512 * hh, 512 * (hh + 1))
        in_maps.append({
            "xT": np.ascontiguousarray(np.asarray(x)[b].T).astype(bf),
            "wqT": np.ascontiguousarray(np.asarray(Wq)[cols, :].T).astype(bf),
            "wkT": np.ascontiguousarray(np.asarray(Wk)[cols, :].T).astype(bf),
            "wvT": np.ascontiguousarray(np.asarray(Wv)[cols, :].T).astype(bf),
            "woT": np.ascontiguousarray(np.asarray(Wo)[:, cols].T).astype(bf),
            "cosT": cosT, "sinT": sinT, "swapT": swapT, "maskT": maskT,
            "eT": eT,
        })
    return in_maps


def kernel(x, Wq, Wk, Wv, Wo, cos_emb, sin_emb, token_positions, **run_kwargs):
    if "nc" not in _compiled:
        _compiled["nc"] = _build_nc()
    nc = _compiled["nc"]
    in_maps = _prep_inputs(x, Wq, Wk, Wv, Wo, cos_emb, sin_emb, token_positions)
    res = run_bass_kernel_spmd(nc, in_maps, list(range(NCORES)), **run_kwargs)
    out = np.empty((B, S, D), np.float32)
    for b in range(B):
        ye = np.asarray(res.results[2 * b]["y"]).astype(np.float32)
        yo = np.asarray(res.results[2 * b + 1]["y"]).astype(np.float32)
        for m in range(NCH):
            for p in range(2):
                g0 = 512 * m + 256 * p
                l0 = 256 * m + 128 * p
                out[b, g0:g0 + 128] = ye[l0:l0 + 128]
                out[b, g0 + 128:g0 + 256] = yo[l0:l0 + 128]
    if run_kwargs:
        kernel.last_result = res
    return out


# revision 44
# speedup vs baseline: 1.1919x; 1.1919x over previous
"""Causal multi-head self-attention with RoPE on 8 NeuronCores.

Sharding: batch (4) x head-halves (2) -> 8 cores. Core c handles batch c//2,
heads [8*(c%2), 8*(c%2)+8). Fully software-pipelined: QKV projection chunks,
attention chunks and the output projection are interleaved in emission order
so the PE never starves while the scalar engine works through the softmax
exps. Output projection is row-sharded over Wo with a per-chunk pair
ReduceScatter writing disjoint row-shards of y on each core; the host
interleaves the shards.
"""

import numpy as np
import ml_dtypes

import concourse.bacc as bacc
import concourse.bass as bass
import concourse.mybir as mybir
from concourse.tile import TileContext
from concourse.bass_utils import run_bass_kernel_spmd

B, S, D, H = 4, 2048, 1024, 16
HL = 8          # heads per core
DK = 64         # head dim
NCORES = 8
DT = D // 128   # 8 contraction tiles over D
OT = HL * DK // 128   # 4 o-tiles for Q^T/K^T ([128, S] each, 2 heads per tile)
ST = S // 128   # 16 s-tiles
NCH = S // 512  # 4 sequence chunks of 512
# vaug col group per head pair: [v_e(64) | 1 | 0*63 | 1 | v_o(64)].
# The even PV matmul uses cols [0:65] -> psum partitions [0:65] (sums at 64);
# the odd uses cols [65:193] (a full 128-wide block at output base partition
# 0) -> sums land at partition 63, data at 64:128, all partition-aligned for
# the downstream normalization multiplies.
GW = 193

BF16 = mybir.dt.bfloat16
F32 = mybir.dt.float32
NEG = -1.0e9

_compiled = {}


def _build_nc():
    nc = bacc.Bacc("TRN2", target_bir_lowering=False, debug=False,
                   num_devices=NCORES)

    xT = nc.dram_tensor("xT", [D, S], BF16, kind="ExternalInput")
    wqT = nc.dram_tensor("wqT", [D, HL * DK], BF16, kind="ExternalInput")
    wkT = nc.dram_tensor("wkT", [D, HL * DK], BF16, kind="ExternalInput")
    wvT = nc.dram_tensor("wvT", [D, HL * DK], BF16, kind="ExternalInput")
    woT = nc.dram_tensor("woT", [HL * DK, D], BF16, kind="ExternalInput")
    cosT = nc.dram_tensor("cosT", [128, S], BF16, kind="ExternalInput")
    sinT = nc.dram_tensor("sinT", [128, S], BF16, kind="ExternalInput")
    swapT = nc.dram_tensor("swapT", [128, 128], BF16, kind="ExternalInput")
    maskT = nc.dram_tensor("maskT", [128, 128], BF16, kind="ExternalInput")
    eT = nc.dram_tensor("eT", [2, 128], mybir.dt.float16, kind="ExternalInput")
    y = nc.dram_tensor("y", [S // 2, D], BF16, kind="ExternalOutput")

    groups = [[0, 1], [2, 3], [4, 5], [6, 7]]

    with TileContext(nc) as tc:
        with (
            tc.tile_pool(name="big", bufs=1) as big,
            tc.tile_pool(name="qr", bufs=2) as qrp,
            tc.tile_pool(name="otp", bufs=2) as otp,
            tc.tile_pool(name="ptile", bufs=16) as ptile,
            tc.tile_pool(name="work", bufs=2) as work,
            tc.tile_pool(name="ps_s", bufs=2, space="PSUM") as ps_s,
            tc.tile_pool(name="ps_a", bufs=2, space="PSUM") as ps_a,
            tc.tile_pool(name="ps_o", bufs=2, space="PSUM") as ps_o,
            tc.tile_pool(name="dram", bufs=1, space="DRAM") as dram,
        ):
            # ---- input loads: priority order, spread over two queues ----
            # the first emitted work is the chunk-0 K projection + RoPE, so
            # wk, sin/cos/swap must land first on the scalar queue; xT leads
            # the sync queue.
            w_sb = {"q": [], "k": [], "v": []}
            for k in range(DT):
                t = big.tile([128, HL * DK], BF16, tag=f"wk{k}", name=f"wk{k}")
                nc.scalar.dma_start(out=t[:], in_=wkT[128 * k:128 * (k + 1), :])
                w_sb["k"].append(t)
            xT_sb = []
            for k in range(DT):
                t = big.tile([128, S], BF16, tag=f"xT{k}", name=f"xT{k}")
                nc.sync.dma_start(out=t[:], in_=xT[128 * k:128 * (k + 1), :])
                xT_sb.append(t)
            sin_sb = big.tile([128, S], BF16, tag="sin")
            nc.scalar.dma_start(out=sin_sb[:], in_=sinT[:])
            cos_sb = big.tile([128, S], BF16, tag="cos")
            nc.scalar.dma_start(out=cos_sb[:], in_=cosT[:])
            swap_sb = big.tile([128, 128], BF16, tag="swap")
            nc.scalar.dma_start(out=swap_sb[:], in_=swapT[:])
            for k in range(DT):
                t = big.tile([128, HL * DK], BF16, tag=f"wq{k}", name=f"wq{k}")
                nc.scalar.dma_start(out=t[:], in_=wqT[128 * k:128 * (k + 1), :])
                w_sb["q"].append(t)
            for k in range(DT):
                t = big.tile([128, HL * DK], BF16, tag=f"wv{k}", name=f"wv{k}")
                nc.sync.dma_start(out=t[:], in_=wvT[128 * k:128 * (k + 1), :])
                w_sb["v"].append(t)
            mask_sb = big.tile([128, 128], BF16, tag="mask")
            nc.scalar.dma_start(out=mask_sb[:], in_=maskT[:])
            e_sb = big.tile([2, 128], mybir.dt.float16, tag="eT")
            nc.scalar.dma_start(out=e_sb[:], in_=eT[:])
            woT_sb = []
            for k in range(OT):
                t = big.tile([128, D], BF16, tag=f"wo{k}", name=f"wo{k}")
                nc.sync.dma_start(out=t[:], in_=woT[128 * k:128 * (k + 1), :])
                woT_sb.append(t)

            # ---- persistent activation tiles ----
            krot_sb = [big.tile([128, S], BF16, tag=f"kr{t}", name=f"kr{t}")
                       for t in range(OT)]
            vaug_sb = [big.tile([128, OT * GW], BF16, tag=f"va{st}",
                                name=f"va{st}") for st in range(ST)]
            # ones / zero-pad columns of each 193-wide head-pair group
            for st in range(ST):
                va3 = vaug_sb[st][:].rearrange("p (g d) -> p g d", d=GW)
                nc.gpsimd.memset(va3[:, :, DK:DK + 1], 1.0)
                nc.gpsimd.memset(va3[:, :, DK + 1:DK + 64], 0.0)
                nc.gpsimd.memset(va3[:, :, DK + 64:DK + 65], 1.0)

            qr_tiles = {}   # (m, ot) -> tile [128, 512]
            oT_tiles = {}   # (m, ot) -> tile [128, 512]
            ypart = [dram.tile([512, D], BF16, tag=f"ypart{m}", name=f"ypart{m}")
                     for m in range(NCH)]

            # ---- quantum builders (deferred emission closures) ----
            def qk_quantum(wname, ot, n):
                # returns (chain_fn, tail_fn): projection chain, then RoPE
                ps_box = {}

                def chain():
                    ps = ps_a.tile([128, 512], F32, tag="psa", name=f"{wname}{ot}_{n}")
                    for k in range(DT):
                        nc.tensor.matmul(
                            ps[:],
                            lhsT=w_sb[wname][k][:, 128 * ot:128 * (ot + 1)],
                            rhs=xT_sb[k][:, 512 * n:512 * (n + 1)],
                            start=(k == 0), stop=(k == DT - 1),
                        )
                    sl = slice(512 * n, 512 * (n + 1))
                    u = work.tile([128, 512], BF16, tag="u", name="u")
                    nc.vector.tensor_mul(u[:], ps[:], sin_sb[:, sl])
                    t1 = work.tile([128, 512], BF16, tag="t1", name="t1")
                    nc.vector.tensor_mul(t1[:], ps[:], cos_sb[:, sl])
                    ps_box["u"], ps_box["t1"] = u, t1

                def tail():
                    sl = slice(512 * n, 512 * (n + 1))
                    ps2 = ps_a.tile([128, 512], F32, tag="psa", name=f"sw{wname}{ot}_{n}")
                    nc.tensor.matmul(ps2[:], lhsT=swap_sb[:], rhs=ps_box["u"][:],
                                     start=True, stop=True)
                    if wname == "q":
                        dst_t = qrp.tile([128, 512], BF16, tag=f"qr{ot}",
                                         name=f"qr{ot}_{n}")
                        qr_tiles[(n, ot)] = dst_t
                        dst = dst_t[:]
                    else:
                        dst = krot_sb[ot][:, sl]
                    nc.vector.tensor_add(dst, ps_box["t1"][:], ps2[:])

                return chain, tail

            def v_quantum(st):
                def chain():
                    ps = ps_a.tile([128, 512], F32, tag="psa", name=f"v{st}")
                    for k in range(DT):
                        nc.tensor.matmul(
                            ps[:],
                            lhsT=xT_sb[k][:, 128 * st:128 * (st + 1)],
                            rhs=w_sb["v"][k][:],
                            start=(k == 0), stop=(k == DT - 1),
                        )
                    va3 = vaug_sb[st][:].rearrange("p (g d) -> p g d", d=GW)
                    src3 = ps[:].rearrange("p (g d) -> p g d", d=128)
                    nc.vector.tensor_copy(va3[:, :, 0:DK], src3[:, :, 0:DK])
                    nc.vector.tensor_copy(va3[:, :, DK + 65:GW],
                                          src3[:, :, DK:128])
                return chain, None

            def proj_quantum(m, r2, nn):
                def chain():
                    i0 = 512 * m
                    r0 = 128 * r2
                    yp = ps_a.tile([128, 512], F32, tag="psa", name=f"yp{m}_{r2}_{nn}")
                    for k in range(OT):
                        nc.tensor.matmul(
                            yp[:],
                            lhsT=oT_tiles[(m, k)][:, r0:r0 + 128],
                            rhs=woT_sb[k][:, 512 * nn:512 * (nn + 1)],
                            start=(k == 0), stop=(k == OT - 1),
                        )
                    yst = work.tile([128, 512], BF16, tag="yst", name="yst")
                    nc.vector.tensor_copy(yst[:], yp[:])
                    nc.sync.dma_start(
                        out=ypart[m][r0:r0 + 128, 512 * nn:512 * (nn + 1)],
                        in_=yst[:])
                return chain, None

            yred = [dram.tile([256, D], BF16, tag=f"yred{m}", name=f"yred{m}")
                    for m in range(NCH)]

            def rs_quantum(m, piece):
                # each chunk's ReduceScatter is split in two halves so the
                # collective overlaps the projection compute. This core's
                # shard of piece p covers global rows 512m+256p + [0,128)
                # (even cores) or +[128,256) (odd cores); stored at
                # y[256m+128p : 256m+128p+128] and interleaved on the host.
                def chain():
                    sl_in = slice(256 * piece, 256 * piece + 256)
                    sl_red = slice(128 * piece, 128 * piece + 128)
                    sl_y = slice(256 * m + 128 * piece,
                                 256 * m + 128 * piece + 128)
                    nc.gpsimd.collective_compute(
                        "ReduceScatter", mybir.AluOpType.add,
                        replica_groups=groups,
                        ins=[ypart[m][sl_in, :].opt()],
                        outs=[yred[m][sl_red, :].opt()],
                    )
                    nc.sync.dma_start(out=y[sl_y, :], in_=yred[m][sl_red, :])
                return chain, None

            def kvq_quanta(m):
                qs = []
                for ot in range(OT):
                    qs.append(qk_quantum("k", ot, m))
                for ot in range(OT):
                    qs.append(qk_quantum("q", ot, m))
                for st in range(4 * m, 4 * m + 4):
                    qs.append(v_quantum(st))
                return qs

            def proj_quanta(m):
                qs = []
                for r2 in range(4):
                    for nn in range(2):
                        qs.append(proj_quantum(m, r2, nn))
                    if r2 == 1:
                        qs.append(rs_quantum(m, piece=0))
                qs.append(rs_quantum(m, piece=1))
                return qs

            # filler stream with one-step delayed tails (keeps PE fed while
            # DVE produces the rope operands)
            class Fillers:
                def __init__(self, quanta):
                    self.quanta = list(quanta)
                    self.pend = None
                    self.idx = 0

                def emit_one(self):
                    if self.idx < len(self.quanta):
                        chain, tail = self.quanta[self.idx]
                        self.idx += 1
                        chain()
                        if self.pend is not None:
                            self.pend()
                        self.pend = tail
                        return True
                    if self.pend is not None:
                        self.pend()
                        self.pend = None
                        return True
                    return False

                def flush(self):
                    while self.emit_one():
                        pass

            # ---- attention chunk ----
            # norm chains are deferred by one tp section (via pending_norm)
            # so their long cross-engine latency (stage->DMA->reciprocal->
            # DMA->E-matmul) never blocks the PE queue
            pending_norm = [None]

            def attn_chunk(m, fillers):
                i0 = 512 * m
                njb = 4 * (m + 1)
                total_slots = OT * njb
                n_fill = len(fillers.quanta) + 1
                acc = 0.0
                pace = max(0.0, (n_fill - OT)) / total_slots
                for tp in range(OT):
                    pTs = []
                    # PV-e chain tiles; emitted at lag 2 inside the QK loop so
                    # the PE has exp-independent work when the scalar engine
                    # lags
                    o_pse = ps_o.tile([128, 512], F32, tag="pso",
                                      name=f"oe{m}_{tp}")

                    def pv_e(jb):
                        dlt = max(0, 128 * jb - i0)
                        nc.tensor.matmul(
                            o_pse[0:DK + 1, dlt:512],
                            lhsT=vaug_sb[jb][:, GW * tp:GW * tp + DK + 1],
                            rhs=pTs[jb][:, dlt:512],
                            start=(jb == 0), stop=(jb == njb - 1),
                            skip_group_check=True,
                        )

                    for jb in range(njb):
                        j0 = 128 * jb
                        dlt = max(0, j0 - i0)
                        s_ps = ps_s.tile([128, 1024], F32, tag="sps",
                                         name=f"s{m}_{tp}_{jb}")
                        for half, po in ((0, 0), (1, DK)):
                            nc.tensor.matmul(
                                s_ps[:, 512 * half + dlt:512 * (half + 1)],
                                lhsT=krot_sb[tp][po:po + DK, j0:j0 + 128],
                                rhs=qr_tiles[(m, tp)][po:po + DK, dlt:512],
                                start=True, stop=True,
                            )
                        pT = ptile.tile([128, 1024], BF16, tag="pT")
                        nc.scalar.activation(
                            pT[:].rearrange("p (b f) -> p b f", b=2)[:, :, dlt:512],
                            s_ps[:].rearrange("p (b f) -> p b f", b=2)[:, :, dlt:512],
                            mybir.ActivationFunctionType.Exp, scale=0.125)
                        if j0 >= i0:
                            # zero the upper-triangle entries post-exp; keeps
                            # the DVE off the QK->exp critical path
                            p3 = pT[:].rearrange("p (b f) -> p b f", b=2)
                            nc.vector.tensor_mul(
                                p3[:, :, dlt:dlt + 128],
                                p3[:, :, dlt:dlt + 128],
                                mask_sb[:].rearrange("p (b f) -> p b f", b=1)
                                .broadcast_to([128, 2, 128]))
                        pTs.append(pT)
                        if jb >= 2:
                            pv_e(jb - 2)
                        if jb == 2 and pending_norm[0] is not None:
                            pending_norm[0]()
                            pending_norm[0] = None
                        acc += pace
                        while acc >= 1.0:
                            fillers.emit_one()
                            acc -= 1.0
                    pv_e(njb - 2)
                    pv_e(njb - 1)
                    o_pso = ps_o.tile([128, 512], F32, tag="pso",
                                      name=f"oo{m}_{tp}")
                    for jb in range(njb):
                        dlt = max(0, 128 * jb - i0)
                        nc.tensor.matmul(
                            o_pso[:, dlt:512],
                            lhsT=vaug_sb[jb][:, GW * tp + DK + 1:GW * (tp + 1)],
                            rhs=pTs[jb][:, 512 + dlt:1024],
                            start=(jb == 0), stop=(jb == njb - 1),
                        )
                    # move the unnormalized o + sums out of PSUM right away
                    osb_e = work.tile([128, 512], F32, tag="osbe", name="osbe")
                    nc.vector.tensor_copy(osb_e[0:DK + 1, :],
                                          o_pse[0:DK + 1, :])
                    osb_o = work.tile([128, 512], F32, tag="osbo", name="osbo")
                    nc.vector.tensor_copy(osb_o[32:DK, :], o_pso[32:DK, :])
                    nc.vector.tensor_copy(osb_o[DK:128, :], o_pso[DK:128, :])
                    fillers.emit_one()

                    def norm(tp, osb_e, osb_o, mm=m):
                        # stage the 2x512 sums into a [128, 8] view by DMA
                        # (DVE reciprocal cost scales with free-dim size
                        # only), reciprocal there, then broadcast across dims
                        # via a K=2 matmul against the selector E
                        recw = work.tile([128, 8], F32, tag="recw",
                                         name="recw")
                        nc.gpsimd.dma_start(out=recw[:, 0:4],
                                            in_=osb_e[DK:DK + 1, :])
                        nc.gpsimd.dma_start(out=recw[:, 4:8],
                                            in_=osb_o[DK - 1:DK, :])
                        nc.vector.reciprocal(recw[:], recw[:])
                        recwh = work.tile([128, 8], mybir.dt.float16,
                                          tag="recwh", name="recwh")
                        nc.vector.tensor_copy(recwh[:], recw[:])
                        rec2h = work.tile([2, 512], mybir.dt.float16,
                                          tag="rec2h", name="rec2h")
                        nc.gpsimd.dma_start(out=rec2h[0:1, :],
                                            in_=recwh[:, 0:4])
                        nc.gpsimd.dma_start(out=rec2h[1:2, :],
                                            in_=recwh[:, 4:8])
                        rep_ps = ps_a.tile([128, 512], F32, tag="psa",
                                           name=f"rep{mm}_{tp}")
                        nc.tensor.matmul(rep_ps[:], lhsT=e_sb[:],
                                         rhs=rec2h[:], start=True, stop=True)
                        rep_sb = work.tile([128, 512], F32, tag="rep",
                                           name="rep")
                        nc.vector.tensor_copy(rep_sb[:], rep_ps[:])
                        oT_t = otp.tile([128, 512], BF16, tag=f"oT{tp}",
                                        name=f"oT{tp}_{mm}")
                        oT_tiles[(mm, tp)] = oT_t
                        nc.vector.tensor_mul(oT_t[0:DK, :], osb_e[0:DK, :],
                                             rep_sb[0:DK, :])
                        nc.vector.tensor_mul(oT_t[DK:128, :], osb_o[DK:128, :],
                                             rep_sb[DK:128, :])

                    import functools
                    pending_norm[0] = functools.partial(norm, tp, osb_e, osb_o)
                fillers.flush()

            # ---- top-level schedule ----
            f0 = Fillers(kvq_quanta(0))
            f0.flush()
            sched = {
                0: kvq_quanta(1),
                1: kvq_quanta(2) + proj_quanta(0),
                2: kvq_quanta(3) + proj_quanta(1),
                3: proj_quanta(2),
            }
            for m in range(NCH):
                attn_chunk(m, Fillers(sched[m]))
            pending_norm[0]()
            pending_norm[0] = None
            Fillers(proj_quanta(3)).flush()

    nc.compile()
    return nc


def _prep_inputs(x, Wq, Wk, Wv, Wo, cos_emb, sin_emb, token_positions):
    bf = ml_dtypes.bfloat16
    cos_g = np.asarray(cos_emb)[np.asarray(token_positions)]  # [S, DK]
    sin_g = np.asarray(sin_emb)[np.asarray(token_positions)]
    # [128, S]: partition p -> head-dim p % 64
    cosT = np.ascontiguousarray(np.tile(cos_g.T, (2, 1))).astype(bf)
    sinT = np.ascontiguousarray(np.tile(sin_g.T, (2, 1))).astype(bf)
    # rotate-half-interleaved as a matmul: rh = SWAP @ u (per 128-dim tile)
    swap = np.zeros((128, 128), np.float32)
    for j in range(64):
        swap[2 * j, 2 * j + 1] = -1.0
        swap[2 * j + 1, 2 * j] = 1.0
    swapT = np.ascontiguousarray(swap.T).astype(bf)
    # 0/1 causal mask for the diagonal 128x128 block in P^T=[j,i] layout,
    # applied multiplicatively after the exp
    jj = np.arange(128)[:, None]
    ii = np.arange(128)[None, :]
    maskT = np.where(ii >= jj, 1.0, 0.0).astype(bf)
    # selector for broadcasting the two per-head reciprocals across dims
    eT = np.zeros((2, 128), np.float16)
    eT[0, 0:64] = 1.0
    eT[1, 64:128] = 1.0

    in_maps = []
    for c in range(NCORES):
        b, hh = c // 2, c % 2
        cols = slice(512 * hh, 512 * (hh + 1))
        in_maps.append({
            "xT": np.ascontiguousarray(np.asarray(x)[b].T).astype(bf),
            "wqT": np.ascontiguousarray(np.asarray(Wq)[cols, :].T).astype(bf),
            "wkT": np.ascontiguousarray(np.asarray(Wk)[cols, :].T).astype(bf),
            "wvT": np.ascontiguousarray(np.asarray(Wv)[cols, :].T).astype(bf),
            "woT": np.ascontiguousarray(np.asarray(Wo)[:, cols].T).astype(bf),
            "cosT": cosT, "sinT": sinT, "swapT": swapT, "maskT": maskT,
            "eT": eT,
        })
    return in_maps


def kernel(x, Wq, Wk, Wv, Wo, cos_emb, sin_emb, token_positions, **run_kwargs):
    if "nc" not in _compiled:
        _compiled["nc"] = _build_nc()
    nc = _compiled["nc"]
    in_maps = _prep_inputs(x, Wq, Wk, Wv, Wo, cos_emb, sin_emb, token_positions)
    res = run_bass_kernel_spmd(nc, in_maps, list(range(NCORES)), **run_kwargs)
    out = np.empty((B, S, D), np.float32)
    for b in range(B):
        ye = np.asarray(res.results[2 * b]["y"]).astype(np.float32)
        yo = np.asarray(res.results[2 * b + 1]["y"]).astype(np.float32)
        for m in range(NCH):
            for p in range(2):
                g0 = 512 * m + 256 * p
                l0 = 256 * m + 128 * p
                out[b, g0:g0 + 128] = ye[l0:l0 + 128]
                out[b, g0 + 128:g0 + 256] = yo[l0:l0 + 128]
    if run_kwargs:
        kernel.last_result = res
    return out
